# revision 10
# baseline (speedup 1.0000x reference)
"""BuddingLayer Trainium2 kernel (8-core expert-parallel).

Reference computation (sizes: N = size_in = 8192, O = size_out = 8192):
    sat_f = saturated.astype(f32)
    mask  = (x * sat_f != 0)                       # active buds
    h1    = relu(rowsum_i(W1[n,j,i]) * x[n]/3 + b1[n,j])        # [N,3]
    h2    = relu(sum_i W2[n,j,i] * h1[n,i] + b2[n,j])           # [N,3]
    h3    = relu(sum_i W3[n,o,i] * h2[n,i] + b3[n,o])           # [N,O]
    u[o]  = sum_n mask[n] * h3[n,o]
    out   = weight @ (x - x*sat_f) + bias + u

Sharding: the n (neuron/expert) axis of the bud stacks (W1,b1,W2,b2,W3,b3,
mask) is split 8 ways; each core produces a partial u over all O outputs.
The dense weight is sharded row-wise over size_out; each core computes its
1024 rows of weight @ x_masked + bias.  Host sums the u partials and adds.

Per-core device layout (n_local = t*128 + p, p = SBUF partition):
  x_own/sat_own  [128, 8]          w1/b1/w2/b2   [128, 8, 3(,3)]
  x_full/sat_full[128, 64]         (k = c*128 + p)
  w3 [1024, 8192, 3] (HBM view)    b3 [1024, 8192] (HBM view)
  wt [8192, 1024] = weight_shard.T bias [1, 1024]
The big stream (w3 + b3 + wt ~ 168 MB/core) is the memory roofline.
"""

import sys

import numpy as np

_TRN = "/opt/trn_rl_repo"
if _TRN not in sys.path:
    sys.path.insert(0, _TRN)

import concourse.bacc as bacc
import concourse.mybir as mybir
from concourse import tile
from concourse.bass_utils import run_bass_kernel_spmd

F32 = mybir.dt.float32
F32R = mybir.dt.float32r
U8 = mybir.dt.uint8
AF = mybir.ActivationFunctionType
ALU = mybir.AluOpType
AX = mybir.AxisListType

N_CORES = 8
SIZE_IN = 8192
SIZE_OUT = 8192


def build_program(
    size_in=SIZE_IN,
    size_out=SIZE_OUT,
    n_cores=N_CORES,
    o_blk=2048,
    w3_bufs=4,
    b3_bufs=3,
    wt_bufs=3,
    bud_dt=None,
    dense_dt=None,
    round_trick=False,
    w3_planes=True,
    enable_asserts=False,
):
    """Build the per-core Bass/Tile program (identical across cores)."""
    n_own = size_in // n_cores      # bud experts owned by this core
    m_own = size_out // n_cores     # dense output rows owned by this core
    nsub = n_own // 128             # 128-partition subtiles of the n shard
    n_ob = size_out // o_blk        # output blocks for the bud stream
    n_kc = size_in // 128           # 128-wide contraction chunks for dense
    n_mb = (m_own + 511) // 512     # 512-wide moving blocks for dense
    steps = n_ob * nsub
    if bud_dt is None:
        bud_dt = F32
    if dense_dt is None:
        dense_dt = F32R
    relu_scale = float(1.0 + 2.0**-14) if (bud_dt == F32R and round_trick) else 1.0

    nc = bacc.Bacc(
        "TRN2",
        target_bir_lowering=False,
        debug=False,
        enable_asserts=enable_asserts,
        num_devices=n_cores,
    )

    d = {}
    d["x_own"] = nc.dram_tensor("x_own", [128, nsub], F32, kind="ExternalInput")
    d["sat_own"] = nc.dram_tensor("sat_own", [128, nsub], U8, kind="ExternalInput")
    d["w1"] = nc.dram_tensor("w1", [128, nsub, 3, 3], F32, kind="ExternalInput")
    d["b1"] = nc.dram_tensor("b1", [128, nsub, 3], F32, kind="ExternalInput")
    d["w2"] = nc.dram_tensor("w2", [128, nsub, 3, 3], F32, kind="ExternalInput")
    d["b2"] = nc.dram_tensor("b2", [128, nsub, 3], F32, kind="ExternalInput")
    d["x_full"] = nc.dram_tensor("x_full", [128, n_kc], F32, kind="ExternalInput")
    d["sat_full"] = nc.dram_tensor("sat_full", [128, n_kc], U8, kind="ExternalInput")
    w3_shape = [n_own, 3, size_out] if w3_planes else [n_own, size_out, 3]
    d["w3"] = nc.dram_tensor("w3", w3_shape, F32, kind="ExternalInput")
    d["b3"] = nc.dram_tensor("b3", [n_own, size_out], F32, kind="ExternalInput")
    d["wt"] = nc.dram_tensor("wt", [size_in, m_own], dense_dt, kind="ExternalInput")
    d["bias"] = nc.dram_tensor("bias", [1, m_own], F32, kind="ExternalInput")
    d["u_out"] = nc.dram_tensor("u_out", [1, size_out], F32, kind="ExternalOutput")
    d["dense_out"] = nc.dram_tensor("dense_out", [1, m_own], F32, kind="ExternalOutput")

    with tile.TileContext(nc) as tc:
        with (
            tc.tile_pool(name="const", bufs=1) as cp,
            tc.tile_pool(name="w3p", bufs=w3_bufs) as w3p,
            tc.tile_pool(name="b3p", bufs=b3_bufs) as b3p,
            tc.tile_pool(name="wtp", bufs=wt_bufs) as wtp,
            tc.tile_pool(name="accp", bufs=2) as accp,
            tc.tile_pool(name="rp", bufs=2) as rp,
            tc.tile_pool(name="outp", bufs=2) as outp,
            tc.tile_pool(name="pp", bufs=1, space="PSUM") as pp,
        ):
            # ---- small constant loads -------------------------------------
            x_own = cp.tile([128, nsub], F32)
            nc.gpsimd.dma_start(x_own[:], d["x_own"][:])
            sat_own = cp.tile([128, nsub], U8)
            nc.gpsimd.dma_start(sat_own[:], d["sat_own"][:])
            w1 = cp.tile([128, nsub, 3, 3], F32)
            nc.gpsimd.dma_start(w1[:], d["w1"][:])
            b1 = cp.tile([128, nsub, 3], F32)
            nc.gpsimd.dma_start(b1[:], d["b1"][:])
            w2 = cp.tile([128, nsub, 3, 3], F32)
            nc.gpsimd.dma_start(w2[:], d["w2"][:])
            b2 = cp.tile([128, nsub, 3], F32)
            nc.gpsimd.dma_start(b2[:], d["b2"][:])
            xf = cp.tile([128, n_kc], F32)
            nc.gpsimd.dma_start(xf[:], d["x_full"][:])
            satf_u8 = cp.tile([128, n_kc], U8)
            nc.gpsimd.dma_start(satf_u8[:], d["sat_full"][:])
            bias_sb = cp.tile([1, m_own], F32)
            nc.gpsimd.dma_start(bias_sb[:], d["bias"][:])

            # ---- mask + h2 for the owned n shard --------------------------
            satof = cp.tile([128, nsub], F32)
            nc.vector.tensor_copy(satof[:], sat_own[:])
            xs = cp.tile([128, nsub], F32)
            nc.vector.tensor_tensor(xs[:], x_own[:], satof[:], op=ALU.mult)
            mask = cp.tile([128, nsub], bud_dt)
            nc.vector.tensor_scalar(mask[:], xs[:], 0.0, None, op0=ALU.not_equal)
            h0 = cp.tile([128, nsub], F32)
            nc.vector.tensor_scalar_mul(h0[:], x_own[:], 1.0 / 3.0)
            rs1 = cp.tile([128, nsub, 3], F32)
            nc.vector.tensor_reduce(rs1[:], w1[:], axis=AX.X, op=ALU.add)
            h1 = cp.tile([128, nsub, 3], F32)
            for t in range(nsub):
                nc.vector.scalar_tensor_tensor(
                    h1[:, t, :], rs1[:, t, :], h0[:, t : t + 1], b1[:, t, :],
                    op0=ALU.mult, op1=ALU.add,
                )
            nc.vector.tensor_scalar_max(h1[:], h1[:], 0.0)
            h2 = cp.tile([128, nsub, 3], F32)
            for t in range(nsub):
                nc.vector.scalar_tensor_tensor(
                    h2[:, t, :], w2[:, t, :, 0], h1[:, t, 0:1], b2[:, t, :],
                    op0=ALU.mult, op1=ALU.add,
                )
                for i in (1, 2):
                    nc.vector.scalar_tensor_tensor(
                        h2[:, t, :], w2[:, t, :, i], h1[:, t, i : i + 1], h2[:, t, :],
                        op0=ALU.mult, op1=ALU.add,
                    )
            nc.vector.tensor_scalar_max(h2[:], h2[:], 0.0)

            # ---- x_masked for the dense matvec ----------------------------
            satff = cp.tile([128, n_kc], F32)
            nc.vector.tensor_copy(satff[:], satf_u8[:])
            xsf = cp.tile([128, n_kc], F32)
            nc.vector.tensor_tensor(xsf[:], xf[:], satff[:], op=ALU.mult)
            xm = cp.tile([128, n_kc], dense_dt)
            nc.vector.tensor_tensor(xm[:], xf[:], xsf[:], op=ALU.subtract)

            # ---- main streamed loop ---------------------------------------
            # Output-block schedule: full-size blocks, with the final block
            # tapered so the end-of-stream compute drain (3 stt + relu +
            # matmul + copy on the last tile) is short.
            o_blocks = []
            o_pos = 0
            while o_pos < size_out:
                rem = size_out - o_pos
                if rem > o_blk:
                    o_blocks.append((o_pos, o_blk))
                    o_pos += o_blk
                elif rem == o_blk and o_blk >= 2048:
                    for ln in (o_blk // 2, o_blk // 4, o_blk // 4):
                        o_blocks.append((o_pos, ln))
                        o_pos += ln
                else:
                    o_blocks.append((o_pos, rem))
                    o_pos += rem
            steps = len(o_blocks) * nsub
            # Dense kc chunks are front-loaded over the first ~3/4 of steps
            # so the dense epilogue never extends the kernel tail.
            ksteps = max(1, (steps * 3) // 4)

            d_psum = pp.tile([1, m_own], F32, tag="dpsum")
            step = 0
            for ob, (o0, o_len) in enumerate(o_blocks):
                u_psum = pp.tile([1, o_len], F32, tag="upsum")
                for t in range(nsub):
                    if w3_planes:
                        w3t = w3p.tile([128, 3, o_len], F32, tag="w3t")
                        nc.sync.dma_start(
                            w3t[:],
                            d["w3"][t * 128 : (t + 1) * 128, :, o0 : o0 + o_len],
                        )
                        w3sl = [w3t[:, i, :] for i in range(3)]
                    else:
                        w3t = w3p.tile([128, o_len, 3], F32, tag="w3t")
                        nc.sync.dma_start(
                            w3t[:],
                            d["w3"][t * 128 : (t + 1) * 128, o0 : o0 + o_len, :],
                        )
                        w3sl = [w3t[:, :, i] for i in range(3)]
                    b3t = b3p.tile([128, o_len], F32, tag="b3t")
                    nc.scalar.dma_start(
                        b3t[:],
                        d["b3"][t * 128 : (t + 1) * 128, o0 : o0 + o_len],
                    )
                    acc = accp.tile([128, o_len], F32, tag="acc")
                    nc.vector.scalar_tensor_tensor(
                        acc[:], w3sl[0], h2[:, t, 0:1], b3t[:],
                        op0=ALU.mult, op1=ALU.add,
                    )
                    for i in (1, 2):
                        nc.vector.scalar_tensor_tensor(
                            acc[:], w3sl[i], h2[:, t, i : i + 1], acc[:],
                            op0=ALU.mult, op1=ALU.add,
                        )
                    r = rp.tile([128, o_len], bud_dt, tag="r")
                    nc.scalar.activation(r[:], acc[:], AF.Relu, scale=relu_scale)
                    for j in range((o_len + 511) // 512):
                        lo, hi = j * 512, min((j + 1) * 512, o_len)
                        nc.tensor.matmul(
                            u_psum[0:1, lo:hi],
                            mask[:, t : t + 1],
                            r[:, lo:hi],
                            start=(t == 0),
                            stop=(t == nsub - 1),
                        )
                    # interleave this step's share of the dense matvec
                    if step < ksteps:
                        for kc in range(
                            step * n_kc // ksteps, (step + 1) * n_kc // ksteps
                        ):
                            wtt = wtp.tile([128, m_own], dense_dt, tag="wtt")
                            nc.scalar.dma_start(
                                wtt[:], d["wt"][kc * 128 : (kc + 1) * 128, :]
                            )
                            for mb in range(n_mb):
                                lo, hi = mb * 512, min((mb + 1) * 512, m_own)
                                nc.tensor.matmul(
                                    d_psum[0:1, lo:hi],
                                    xm[:, kc : kc + 1],
                                    wtt[:, lo:hi],
                                    start=(kc == 0),
                                    stop=(kc == n_kc - 1),
                                )
                        if step == ksteps - 1:
                            dense_sb = outp.tile([1, m_own], F32, tag="dense_sb")
                            nc.vector.tensor_tensor(
                                dense_sb[:], d_psum[:], bias_sb[:], op=ALU.add
                            )
                            nc.gpsimd.dma_start(d["dense_out"][:], dense_sb[:])
                    step += 1
                u_sb = outp.tile([1, o_len], F32, tag="u_sb")
                nc.vector.tensor_copy(u_sb[:], u_psum[:])
                nc.gpsimd.dma_start(d["u_out"][0:1, o0 : o0 + o_len], u_sb[:])

    nc.compile()
    return nc, d


def make_in_maps(inputs, size_in=SIZE_IN, size_out=SIZE_OUT, n_cores=N_CORES,
                 w3_planes=True):
    """Shard + re-layout the full inputs into one in_map per core."""
    x = np.ascontiguousarray(np.asarray(inputs["x"], dtype=np.float32))
    sat = np.asarray(inputs["saturated"]).astype(np.uint8)
    weight = np.asarray(inputs["weight"], dtype=np.float32)
    bias = np.asarray(inputs["bias"], dtype=np.float32)
    W1 = np.asarray(inputs["W1"], dtype=np.float32)
    b1 = np.asarray(inputs["b1"], dtype=np.float32)
    W2 = np.asarray(inputs["W2"], dtype=np.float32)
    b2 = np.asarray(inputs["b2"], dtype=np.float32)
    W3 = np.asarray(inputs["W3"], dtype=np.float32)
    b3 = np.asarray(inputs["b3"], dtype=np.float32)

    n_own = size_in // n_cores
    m_own = size_out // n_cores
    nsub = n_own // 128
    n_kc = size_in // 128

    x_full = np.ascontiguousarray(x.reshape(n_kc, 128).T)
    sat_full = np.ascontiguousarray(sat.reshape(n_kc, 128).T)

    in_maps = []
    for i in range(n_cores):
        sl = slice(i * n_own, (i + 1) * n_own)
        slm = slice(i * m_own, (i + 1) * m_own)
        m = {
            "x_own": np.ascontiguousarray(x[sl].reshape(nsub, 128).T),
            "sat_own": np.ascontiguousarray(sat[sl].reshape(nsub, 128).T),
            "w1": np.ascontiguousarray(
                W1[sl].reshape(nsub, 128, 3, 3).transpose(1, 0, 2, 3)
            ),
            "b1": np.ascontiguousarray(
                b1[sl].reshape(nsub, 128, 3).transpose(1, 0, 2)
            ),
            "w2": np.ascontiguousarray(
                W2[sl].reshape(nsub, 128, 3, 3).transpose(1, 0, 2, 3)
            ),
            "b2": np.ascontiguousarray(
                b2[sl].reshape(nsub, 128, 3).transpose(1, 0, 2)
            ),
            "x_full": x_full,
            "sat_full": sat_full,
            "w3": (np.ascontiguousarray(W3[sl].transpose(0, 2, 1))
                    if w3_planes else W3[sl]),
            "b3": b3[sl],
            "wt": np.ascontiguousarray(weight[slm].T),
            "bias": bias[slm].reshape(1, m_own),
        }
        in_maps.append(m)
    return in_maps


def combine_outputs(results, names, size_out=SIZE_OUT):
    """Gather/unshard: sum u partials, concat dense rows, add."""
    u = np.zeros(size_out, dtype=np.float64)
    dense = []
    for res in results:
        u += res[names["u_out"].name].reshape(-1).astype(np.float64)
        dense.append(res[names["dense_out"].name].reshape(-1))
    out = np.concatenate(dense).astype(np.float64) + u
    return out.astype(np.float32)


_CACHE = {}
CONFIG = {}


def _get_program():
    if "nc" not in _CACHE:
        _CACHE["nc"], _CACHE["names"] = build_program(**CONFIG)
    return _CACHE["nc"], _CACHE["names"]


def kernel(**inputs):
    nc, names = _get_program()
    in_maps = make_in_maps(inputs)
    keyed = [
        {names[k].name: v for k, v in m.items()} for m in in_maps
    ]
    res = run_bass_kernel_spmd(nc, keyed, core_ids=list(range(N_CORES)))
    return combine_outputs(res.results, names)
